# revision 52
# baseline (speedup 1.0000x reference)
"""Trainium2 Bass kernel for nn_MultiHeadAttention_9036611191413 (v2).

Reference computation (B=4, S=2048, D_IN=512, H=8, D_K=64):
    qh = (q @ Wq)  -> [B,H,S,64]   (split heads); kh, vh likewise
    scores = qh @ kh^T / 8;  scores = where(scores>0, scores, -1e4)
    attn = softmax(scores); out = attn @ vh -> merge heads -> @ Wo
    result = LayerNorm(q + out)

Sharding: 8 cores = (batch b, query-half).  Each core owns 1024 query rows of
one batch, all 8 heads; K/V work duplicated across the 2 cores of a batch.

Design (v2):
  - Inputs transposed on the PE (identity matmul) instead of a DRAM bounce.
  - Projections in fp8(e4m3) with DoubleRow (2 k-tiles per pass).
  - Scores in bf16, K=64 per head, F=1024 (full query block per core).
  - exp on ACT with scale=1/8, bias=-7 writing fp8 e4m3: weights for
    scores<~0.07 fall below the e4m3 subnormal range and flush to 0,
    implementing the where(s>0) threshold; e^-7 scaling cancels in softmax.
  - attn@V in fp8 DoubleRow over key-chunk pairs; softmax denominator via a
    ones column in V~.
  - 1/D via DVE stream-transpose + Quake-initialized Newton (no ACT table
    switches: ACT runs Exp only).
  - LayerNorm rstd via Newton rsqrt on DVE.
"""

import os
import sys
import numpy as np

try:
    import concourse.bass as bass
except ImportError:  # fresh grading dir: point at the repo checkout
    for p in ("/opt/trn_rl_repo", "/root/.axon_site/_ro/trn_rl_repo"):
        if os.path.isdir(p):
            sys.path.insert(0, p)
    import concourse.bass as bass

import concourse.mybir as mybir
import concourse.tile as tile
from concourse import bacc
from concourse.bass_utils import run_bass_kernel_spmd
from concourse.masks import make_identity
from contextlib import ExitStack

FP32 = mybir.dt.float32
BF16 = mybir.dt.bfloat16
FP8 = mybir.dt.float8e4
I32 = mybir.dt.int32
AF = mybir.ActivationFunctionType
OP = mybir.AluOpType
DR = mybir.MatmulPerfMode.DoubleRow

B, S, DIN, H, DK = 4, 2048, 512, 8, 64
DM = H * DK            # 512
SQ = S // 2            # 1024 query rows per core
NCORES = 8
EPS = 1e-5
C_EXP = 7.0            # exp bias: p = exp(s/8 - 7); e4m3 FTZ applies threshold

NT_Q = SQ // 128       # 8   query token tiles
NT_K = S // 128        # 16  key token tiles
NIC = DIN // 128       # 4   input-dim chunks
NDC = DM // 128        # 4   d_model chunks (2 heads per chunk)
VW = 72                # Vt8 padded head stride (65 used, 16B-aligned pairs)

MAGIC_RECIP = 0x7EF311C3
MAGIC_RSQRT = 0x5F3759DF


def build_program():
    nc = bacc.Bacc("TRN2", target_bir_lowering=False, debug=False)

    q_d = nc.dram_tensor("q", [SQ, DIN], FP32, kind="ExternalInput")
    k_d = nc.dram_tensor("k", [S, DIN], FP32, kind="ExternalInput")
    v_d = nc.dram_tensor("v", [S, DIN], FP32, kind="ExternalInput")
    wq_d = nc.dram_tensor("wq", [DIN, DM], FP32, kind="ExternalInput")
    wk_d = nc.dram_tensor("wk", [DIN, DM], FP32, kind="ExternalInput")
    wv_d = nc.dram_tensor("wv", [DIN, DM], FP32, kind="ExternalInput")
    wo_d = nc.dram_tensor("wo", [DM, DIN], FP32, kind="ExternalInput")
    out_d = nc.dram_tensor("out", [SQ, DIN], FP32, kind="ExternalOutput")

    with tile.TileContext(nc) as tc, ExitStack() as ctx:
        const = ctx.enter_context(tc.tile_pool(name="const", bufs=1))
        wpool = ctx.enter_context(tc.tile_pool(name="wpool", bufs=1))
        resid = ctx.enter_context(tc.tile_pool(name="resid", bufs=1))
        xTp = ctx.enter_context(tc.tile_pool(name="xTp", bufs=1))
        projp = ctx.enter_context(tc.tile_pool(name="projp", bufs=1))
        attnp = ctx.enter_context(tc.tile_pool(name="attnp", bufs=1))
        epool = ctx.enter_context(tc.tile_pool(name="epool", bufs=4))
        outp = ctx.enter_context(tc.tile_pool(name="outp", bufs=3))

        # --- constants ---
        ident_bf = const.tile([128, 128], BF16, tag="identbf")
        make_identity(nc, ident_bf[:])
        cexp_t = const.tile([128, 1], FP32, tag="cexp")
        nc.gpsimd.memset(cexp_t[:], -C_EXP)
        ones64 = const.tile([64, 64], BF16, tag="ones64")
        nc.gpsimd.memset(ones64[:], 1.0)

        # --- phase 0: load, cast bf16, PE-transpose, store fp8 ---
        # transposed inputs, fp8, [i-part(128), ic, tokens]
        qT8 = xTp.tile([128, NIC, SQ], FP8, tag="qT8")
        kT8 = xTp.tile([128, NIC, S], FP8, tag="kT8")
        vT8 = xTp.tile([128, NIC, S], FP8, tag="vT8")
        q_all = resid.tile([128, NT_Q, DIN], FP32, tag="qresid")

        w8 = {}
        ps2 = ctx.enter_context(tc.tile_pool(name="ps2", bufs=2, space="PSUM"))
        phase2 = ExitStack()
        pso = phase2.enter_context(
            tc.tile_pool(name="pso", bufs=1, space="PSUM"))
        psr_box = []
        phase1 = ExitStack()
        stage = phase1.enter_context(tc.tile_pool(name="stage", bufs=1))
        phase1a = ExitStack()
        stage1 = phase1a.enter_context(tc.tile_pool(name="stage1", bufs=1))
        pt_ps = phase1a.enter_context(
            tc.tile_pool(name="ptps", bufs=2, space="PSUM"))
        psj_box = []

        def load_w8(name, wd):
            wst = stage.tile([128, NIC, DM], FP32, tag="wst", bufs=1,
                             name=f"{name}st")
            nc.sync.dma_start(
                wst[:], wd[:, :].rearrange("(ic p) d -> p ic d", p=128))
            wb = wpool.tile([128, NIC, DM], FP8, tag=f"{name}8",
                            name=f"{name}8")
            nc.vector.tensor_copy(wb[:], wst[:])
            w8[name] = wb

        def trans_tiles(src_bf, dst8, tt0, ntt):
            # src_bf [128, ntt, 512] bf16 token tiles -> dst8 [128, NIC, S]
            for t in range(ntt):
                pt = pt_ps.tile([128, NIC, 128], BF16, tag="pt", name="pt")
                for ic in range(NIC):
                    nc.tensor.transpose(
                        pt[:, ic, :], src_bf[:, t, ic * 128:(ic + 1) * 128],
                        ident_bf[:])
                nc.vector.tensor_copy(
                    dst8[:, :, (tt0 + t) * 128:(tt0 + t + 1) * 128], pt[:])

        # dk-split DoubleRow layouts: [32, head, dk-half, tokens]
        QT8i = projp.tile([32, H, 2, SQ], FP8, tag="QT8i")
        KT8i = projp.tile([32, H, 2, S], FP8, tag="KT8i")
        Vt8 = projp.tile([128, NT_K, H, VW], FP8, tag="Vt8")
        nc.gpsimd.memset(Vt8[:, :, :, DK:DK + 1], 1.0)

        def proj_half(w8t, rhs8, dc, tok0):
            # one [128, 512] psum half of a DoubleRow projection
            pj = proj_psum()
            for j in range(2):
                nc.tensor.matmul(
                    pj[:], w8t[:, 2 * j:2 * j + 2, dc * 128:(dc + 1) * 128],
                    rhs8[:, 2 * j:2 * j + 2, tok0:tok0 + 512],
                    start=(j == 0), stop=(j == 1), perf_mode=DR)
            return pj

        def proj_psum():
            if psj_box:
                return psj_box[0].tile([128, 512], FP32, tag="psj",
                                       bufs=2, name="pj")
            return ps2.tile([128, 1024], FP32, tag="ps2", name="pj")[:, 0:512]

        def k_proj_kb(dc, kb, kstg):
            for n in range(2):
                pj = proj_half(w8["wk"], kT8, dc, kb * 1024 + n * 512)
                nc.vector.tensor_copy(
                    kstg[:, kb * 1024 + n * 512:kb * 1024 + (n + 1) * 512],
                    pj[:])

        def k_proj_fin(dc, kstg):
            for hh in range(2):
                for r in range(2):
                    nc.sync.dma_start(
                        KT8i[:, 2 * dc + hh, r, :],
                        kstg[hh * 64 + r * 32:hh * 64 + (r + 1) * 32, :])

        def k_proj(dc):
            kstg = stage.tile([128, S], FP8, tag="kstg", bufs=2, name="kstg")
            for kb in range(2):
                k_proj_kb(dc, kb, kstg)
            k_proj_fin(dc, kstg)

        def q_proj(dc):
            qstg = stage.tile([128, SQ], FP8, tag="qstg", bufs=2, name="qstg")
            for n in range(2):
                pj = proj_half(w8["wq"], qT8, dc, n * 512)
                nc.vector.tensor_copy(
                    qstg[:, n * 512:(n + 1) * 512], pj[:])
            for hh in range(2):
                for r in range(2):
                    nc.sync.dma_start(
                        QT8i[:, 2 * dc + hh, r, :],
                        qstg[hh * 64 + r * 32:hh * 64 + (r + 1) * 32, :])

        def kq_proj(dc):
            k_proj(dc)
            q_proj(dc)

        def v_proj(tt):
            # V natural [tokens, dm] into the per-head 72-padded layout
            pv = proj_psum()
            for j in range(2):
                nc.tensor.matmul(
                    pv[:], vT8[:, 2 * j:2 * j + 2, tt * 128:(tt + 1) * 128],
                    w8["wv"][:, 2 * j:2 * j + 2, :],
                    start=(j == 0), stop=(j == 1), perf_mode=DR)
            nc.vector.tensor_copy(
                Vt8[:, tt, :, 0:DK],
                pv.rearrange("p (h d) -> p h d", d=DK))

        # k chain first: K-projection heads the attention pipeline
        load_w8("wk", wk_d)
        kbf_chunks = []
        for c in range(4):
            rows = slice(c * 4 * 128, (c + 1) * 4 * 128)
            ldc = stage1.tile([128, 4, DIN], FP32, tag="ldc", bufs=3,
                             name=f"kld{c}")
            nc.sync.dma_start(
                ldc[:], k_d[rows, :].rearrange("(tt p) i -> p tt i", p=128))
            cbf = stage1.tile([128, 4, DIN], BF16, tag="cbf", bufs=3,
                             name=f"kbf{c}")
            nc.vector.tensor_copy(cbf[:], ldc[:])
            trans_tiles(cbf, kT8, c * 4, 4)
        # q chain: q DMA queued right after k so the first scores fire early
        load_w8("wq", wq_d)
        nc.sync.dma_start(
            q_all[:], q_d[:, :].rearrange("(tt p) i -> p tt i", p=128))
        # v DMAs after q in the queues
        load_w8("wv", wv_d)
        vlds = []
        for c in range(4):
            rows = slice(c * 4 * 128, (c + 1) * 4 * 128)
            vld = stage1.tile([128, 4, DIN], FP32, tag="ldc", bufs=3,
                              name=f"vld{c}")
            nc.sync.dma_start(
                vld[:], v_d[rows, :].rearrange("(tt p) i -> p tt i", p=128))
            vlds.append(vld)
        k_proj(0)
        qbf = stage1.tile([128, NT_Q, DIN], BF16, tag="qbf")
        nc.vector.tensor_copy(qbf[:], q_all[:])
        trans_tiles(qbf, qT8, 0, NT_Q)
        q_proj(0)

        def v_chain():
            for c in range(4):
                cbf = stage1.tile([128, 4, DIN], BF16, tag="cbf", bufs=3,
                                  name=f"vbf{c}")
                nc.vector.tensor_copy(cbf[:], vlds[c][:])
                trans_tiles(cbf, vT8, c * 4, 4)
            phase1a.close()  # free ldc/cbf/qbf SBUF + transpose PSUM banks
            psj_box.append(phase1.enter_context(
                tc.tile_pool(name="psj", bufs=1, space="PSUM")))

        # deferred phase-0/1 work, drip-fed into the attention loop
        def wo_load():
            wost = stage.tile([128, NDC, DIN], FP32, tag="wst", bufs=1)
            nc.sync.dma_start(
                wost[:], wo_d[:, :].rearrange("(dc p) d -> p dc d", p=128))
            wo8_t = wpool.tile([128, NDC, DIN], FP8, tag="wo8", name="wo8")
            nc.vector.tensor_copy(wo8_t[:], wost[:])
            wo8_box.append(wo8_t)

        wo8_box = []
        pending = []
        for t in range(NT_K):
            pending.append(lambda t=t: v_proj(t))
        for dcx in (1, 2, 3):
            kst_box = {}

            def mk(dcx=dcx, kst_box=kst_box):
                def kb0():
                    kst_box['t'] = stage.tile([128, S], FP8, tag="kstg",
                                              bufs=2, name="kstg")
                    k_proj_kb(dcx, 0, kst_box['t'])

                def kb1():
                    k_proj_kb(dcx, 1, kst_box['t'])
                    k_proj_fin(dcx, kst_box['t'])
                return kb0, kb1
            kb0, kb1 = mk()
            pending.append(kb0)
            pending.append(kb1)
            pending.append(lambda dcx=dcx: q_proj(dcx))
        pending.append(wo_load)

        def close_phase1():
            phase1.close()  # frees stage/psj; then the norm-phase PSUM pool
            psr_box.append(phase2.enter_context(
                tc.tile_pool(name="psr", bufs=1, space="PSUM")))
        pending.append(close_phase1)

        # --- phase 2: attention ---
        # Dsb2: D-row of the pair's first head at partition 0, second at 32
        Dsb2 = attnp.tile([64, SQ], BF16, tag="Dsb2")
        nc.gpsimd.memset(Dsb2[:], 1.0)
        Dt = attnp.tile([128, NT_Q, 2, 32], BF16, tag="Dt")
        Dtf = attnp.tile([128, NT_Q, 2, 1], FP32, tag="Dtf")
        yA = attnp.tile([128, NT_Q, 2, 1], FP32, tag="yA")
        yB = attnp.tile([128, NT_Q, 2, 1], FP32, tag="yB")
        rt_bf = attnp.tile([128, NT_Q, 2, 32], BF16, tag="rtbf")
        rrow2 = attnp.tile([64, NT_Q, 4, 32], BF16, tag="rrow2")
        OT8 = attnp.tile([128, NDC, SQ], FP8, tag="OT8")

        def norm_a(dc):
            # DVE-only: transpose D rows, Newton reciprocal, transpose back
            dsrc = Dsb2.rearrange("p (c im) -> p c im", im=128)
            for X in range(2):
                for i in range(4):
                    nc.vector.transpose(
                        Dt[32 * i:32 * (i + 1), :, X, :],
                        dsrc[32 * X:32 * X + 32, :, 32 * i:32 * (i + 1)])
            sl = (slice(None), slice(None), slice(None), slice(0, 1))
            nc.vector.tensor_copy(Dtf[:], Dt[sl])
            d_i = Dtf.bitcast(I32)
            y_i = yA.bitcast(I32)
            # y0 = bitcast(MAGIC - bits(D)) = bitcast(~bits(D) + MAGIC + 1)
            nc.vector.tensor_scalar(
                out=y_i[:], in0=d_i[:], scalar1=0, op0=OP.bitwise_not,
                scalar2=0, op1=OP.bypass)
            nc.vector.tensor_scalar(
                out=y_i[:], in0=y_i[:], scalar1=MAGIC_RECIP + 1, op0=OP.add,
                scalar2=0, op1=OP.bypass)
            # two Newton steps, tracking m = -y to avoid reverse-subtract
            nc.vector.tensor_tensor(out=yB[:], in0=Dtf[:], in1=yA[:],
                                    op=OP.mult)
            nc.vector.scalar_tensor_tensor(
                out=yB[:], in0=yB[:], scalar=2.0, in1=yA[:],
                op0=OP.subtract, op1=OP.mult)  # m1 = (Dy0-2)y0 = -y1
            nc.vector.tensor_tensor(out=yA[:], in0=Dtf[:], in1=yB[:],
                                    op=OP.mult)  # u2 = -D*y1
            nc.vector.scalar_tensor_tensor(
                out=yB[:], in0=yA[:], scalar=2.0, in1=yB[:],
                op0=OP.add, op1=OP.mult)  # m2 = (2-Dy1)(-y1) = -1/D
            nc.vector.tensor_scalar(
                out=rt_bf[sl], in0=yB[:], scalar1=-1.0, op0=OP.mult,
                scalar2=0.0, op1=OP.bypass)
            for X in range(2):
                for i in range(4):
                    nc.vector.transpose(
                        rrow2[32 * X:32 * X + 32, :, i, :],
                        rt_bf[32 * i:32 * i + 32, :, X, :])

        def norm_b(dc, poSB):
            # PE broadcast of 1/D + the OT multiply (deferred 2 pair-slots)
            rrow_flat = rrow2.rearrange("p c i m -> p (c i m)")
            for hh in range(2):
                oth = attnp.tile([64, SQ], FP8, tag="oth", bufs=2, name="oth")
                for n in range(2):
                    rrep = psr_box[0].tile([64, 512], FP32, tag="psr",
                                           bufs=2, name="rrep")
                    nc.tensor.matmul(
                        rrep[:], ones64[32 * hh:32 * hh + 1, :],
                        rrow_flat[32 * hh:32 * hh + 1,
                                  n * 512:(n + 1) * 512],
                        start=True, stop=True)
                    nc.vector.tensor_tensor(
                        out=oth[:, n * 512:(n + 1) * 512],
                        in0=poSB[hh][0:DK, n * 512:(n + 1) * 512],
                        in1=rrep[:], op=OP.mult)
                nc.sync.dma_start(OT8[hh * 64:(hh + 1) * 64, dc, :], oth[:])

        NPAIR = NT_K // 2  # 8 key-chunk pairs per head
        poSB_pair = [None, None]
        delayed = []   # (due_g, fn)

        def evac_head(ph, ppo, gnow):
            # po PSUM -> SBUF (O rows + D row), D row -> Dsb2 via DMA
            pdc, phh = ph // 2, ph % 2
            poSB = attnp.tile([DK + 1, SQ], BF16, tag="poSB", bufs=2,
                              name="poSB")
            nc.vector.tensor_copy(poSB[:], ppo[:])
            nc.sync.dma_start(Dsb2[32 * phh:32 * phh + 1, :],
                              poSB[DK:DK + 1, :])
            poSB_pair[phh] = poSB
            if phh == 1:
                norm_a(pdc)
                pair = list(poSB_pair)
                delayed.append((gnow + 4, lambda: norm_b(pdc, pair)))

        inflight = []      # [(h, j, e_pair, po)] attnV lags 2 pairs

        def do_av(ph, pj, pe, ppo, gnow):
            for n in range(2):
                nc.tensor.matmul(
                    ppo[:, n * 512:(n + 1) * 512],
                    Vt8[:, 2 * pj:2 * pj + 2, ph, 0:DK + 1],
                    pe[:, :, n * 512:(n + 1) * 512],
                    start=(pj == 0), stop=(pj == NPAIR - 1),
                    perf_mode=DR, skip_group_check=True)
            if pj == NPAIR - 1:
                evac_head(ph, ppo, gnow)

        po = None
        for g in range(H * NPAIR):
            if g == 2:
                v_chain()
            while delayed and delayed[0][0] <= g:
                delayed.pop(0)[1]()
            if g >= 2:
                for _ in range(2):
                    if pending:
                        pending.pop(0)()
            h, j = g // NPAIR, g % NPAIR
            dc, hh = h // 2, h % 2
            if j == 0:
                po = pso.tile([DK + 1, SQ], FP32, tag="pso", name="po")
            e_pair = epool.tile([128, 2, SQ], FP8, tag="e", name="e")
            for sl2 in range(2):
                kc = 2 * j + sl2
                ss = ps2.tile([128, SQ], FP32, tag="ps2", name="ss")
                for n in range(2):
                    nc.tensor.matmul(
                        ss[:, n * 512:(n + 1) * 512],
                        KT8i[:, h, :, kc * 128:(kc + 1) * 128],
                        QT8i[:, h, :, n * 512:(n + 1) * 512],
                        start=True, stop=True, perf_mode=DR)
                nc.scalar.activation(e_pair[:, sl2, :], ss[:], AF.Exp,
                                     bias=cexp_t[:], scale=0.125)
            if len(inflight) >= 2:
                do_av(*inflight.pop(0), g)
            inflight.append((h, j, e_pair, po))
        # drain the remaining pairs
        for item in inflight:
            do_av(*item, 10 ** 9)
        for _, fn in delayed:
            fn()
        delayed = []
        phase2.close()  # free pso/psr PSUM banks before the LN-phase pool

        # --- phase 3: out-projection + residual + LayerNorm ---
        lnp = ctx.enter_context(tc.tile_pool(name="lnp", bufs=1))
        with tc.tile_pool(name="psz", bufs=4, space="PSUM") as psz, \
             tc.tile_pool(name="lns", bufs=2) as lns:
            mvall = lnp.tile([128, NT_Q, 2], FP32, tag="mvall")
            vtmp = lnp.tile([128, NT_Q], FP32, tag="vtmp")
            rsA = lnp.tile([128, NT_Q], FP32, tag="rsA")
            rsB = lnp.tile([128, NT_Q], FP32, tag="rsB")
            x_tiles = []
            for t in range(NT_Q):
                zp = psz.tile([128, DIN], FP32, tag="psz", name="zp")
                for m in range(2):
                    nc.tensor.matmul(
                        zp[:],
                        OT8[:, 2 * m:2 * m + 2, t * 128:(t + 1) * 128],
                        wo8_box[0][:, 2 * m:2 * m + 2, :],
                        start=(m == 0), stop=(m == 1), perf_mode=DR)
                x = lnp.tile([128, DIN], FP32, tag=f"x{t}", name=f"x{t}")
                nc.vector.tensor_tensor(out=x[:], in0=zp[:],
                                        in1=q_all[:, t, :], op=OP.add)
                st = lns.tile([128, 6], FP32, tag="st", name="st")
                nc.vector.bn_stats(st[:], x[:])
                nc.vector.bn_aggr(mvall[:, t, :], st[:])
                x_tiles.append(x)
            # rstd = rsqrt(var + eps) via Quake-Newton on DVE
            nc.vector.tensor_scalar(
                out=vtmp[:], in0=mvall[:, :, 1], scalar1=EPS, op0=OP.add,
                scalar2=0.0, op1=OP.bypass)
            v_i = vtmp.bitcast(I32)
            y_i = rsA.bitcast(I32)
            nc.vector.tensor_scalar(
                out=y_i[:], in0=v_i[:], scalar1=1,
                op0=OP.logical_shift_right, scalar2=0, op1=OP.bitwise_not)
            nc.vector.tensor_scalar(
                out=y_i[:], in0=y_i[:], scalar1=MAGIC_RSQRT + 1, op0=OP.add,
                scalar2=0, op1=OP.bypass)
            for it in range(2):
                nc.vector.tensor_tensor(out=rsB[:], in0=rsA[:], in1=rsA[:],
                                        op=OP.mult)
                nc.vector.tensor_tensor(out=rsB[:], in0=vtmp[:], in1=rsB[:],
                                        op=OP.mult)
                nc.vector.tensor_scalar(
                    out=rsB[:], in0=rsB[:], scalar1=-0.5, op0=OP.mult,
                    scalar2=1.5, op1=OP.add)
                nc.vector.tensor_tensor(out=rsA[:], in0=rsB[:], in1=rsA[:],
                                        op=OP.mult)
            for t in range(NT_Q):
                ot = outp.tile([128, DIN], FP32, tag="oout", name="ot")
                nc.vector.tensor_scalar(
                    out=ot[:], in0=x_tiles[t][:],
                    scalar1=mvall[:, t, 0:1],
                    scalar2=rsA[:, t:t + 1],
                    op0=OP.subtract, op1=OP.mult)
                nc.sync.dma_start(out_d[t * 128:(t + 1) * 128, :], ot[:])

    nc.compile()
    return nc


_PROGRAM = None


def _get_program():
    global _PROGRAM
    if _PROGRAM is None:
        _PROGRAM = build_program()
    return _PROGRAM


def _make_in_maps(q, k, v, Wq, Wk, Wv, Wo):
    in_maps = []
    for c in range(NCORES):
        b, qh = c // 2, c % 2
        in_maps.append({
            "q": np.ascontiguousarray(q[b, qh * SQ:(qh + 1) * SQ, :]),
            "k": np.ascontiguousarray(k[b]),
            "v": np.ascontiguousarray(v[b]),
            "wq": Wq, "wk": Wk, "wv": Wv, "wo": Wo,
        })
    return in_maps


def _assemble(results):
    out = np.empty((B, S, DIN), np.float32)
    for c in range(NCORES):
        b, qh = c // 2, c % 2
        out[b, qh * SQ:(qh + 1) * SQ, :] = results[c]["out"]
    return out


def run(trace=False, **inputs):
    f32 = lambda x: np.asarray(x, dtype=np.float32)
    q, k, v = f32(inputs["q"]), f32(inputs["k"]), f32(inputs["v"])
    Wq, Wk, Wv, Wo = (f32(inputs[n]) for n in ("Wq", "Wk", "Wv", "Wo"))
    nc = _get_program()
    in_maps = _make_in_maps(q, k, v, Wq, Wk, Wv, Wo)
    res = run_bass_kernel_spmd(nc, in_maps, list(range(NCORES)), trace=trace)
    return _assemble(res.results), res.exec_time_ns


def kernel(**inputs):
    out, _ = run(trace=False, **inputs)
    return out


# revision 54
# speedup vs baseline: 1.0064x; 1.0064x over previous
"""Trainium2 Bass kernel for nn_MultiHeadAttention_9036611191413 (v2).

Reference computation (B=4, S=2048, D_IN=512, H=8, D_K=64):
    qh = (q @ Wq)  -> [B,H,S,64]   (split heads); kh, vh likewise
    scores = qh @ kh^T / 8;  scores = where(scores>0, scores, -1e4)
    attn = softmax(scores); out = attn @ vh -> merge heads -> @ Wo
    result = LayerNorm(q + out)

Sharding: 8 cores = (batch b, query-half).  Each core owns 1024 query rows of
one batch, all 8 heads; K/V work duplicated across the 2 cores of a batch.

Design (v2):
  - Inputs transposed on the PE (identity matmul) instead of a DRAM bounce.
  - Projections in fp8(e4m3) with DoubleRow (2 k-tiles per pass).
  - Scores in bf16, K=64 per head, F=1024 (full query block per core).
  - exp on ACT with scale=1/8, bias=-7 writing fp8 e4m3: weights for
    scores<~0.07 fall below the e4m3 subnormal range and flush to 0,
    implementing the where(s>0) threshold; e^-7 scaling cancels in softmax.
  - attn@V in fp8 DoubleRow over key-chunk pairs; softmax denominator via a
    ones column in V~.
  - 1/D via DVE stream-transpose + Quake-initialized Newton (no ACT table
    switches: ACT runs Exp only).
  - LayerNorm rstd via Newton rsqrt on DVE.
"""

import os
import sys
import numpy as np

try:
    import concourse.bass as bass
except ImportError:  # fresh grading dir: point at the repo checkout
    for p in ("/opt/trn_rl_repo", "/root/.axon_site/_ro/trn_rl_repo"):
        if os.path.isdir(p):
            sys.path.insert(0, p)
    import concourse.bass as bass

import concourse.mybir as mybir
import concourse.tile as tile
from concourse import bacc
from concourse.bass_utils import run_bass_kernel_spmd
from concourse.masks import make_identity
from contextlib import ExitStack

FP32 = mybir.dt.float32
BF16 = mybir.dt.bfloat16
FP8 = mybir.dt.float8e4
I32 = mybir.dt.int32
AF = mybir.ActivationFunctionType
OP = mybir.AluOpType
DR = mybir.MatmulPerfMode.DoubleRow

B, S, DIN, H, DK = 4, 2048, 512, 8, 64
DM = H * DK            # 512
SQ = S // 2            # 1024 query rows per core
NCORES = 8
EPS = 1e-5
C_EXP = 7.0            # exp bias: p = exp(s/8 - 7); e4m3 FTZ applies threshold

NT_Q = SQ // 128       # 8   query token tiles
NT_K = S // 128        # 16  key token tiles
NIC = DIN // 128       # 4   input-dim chunks
NDC = DM // 128        # 4   d_model chunks (2 heads per chunk)
VW = 72                # Vt8 padded head stride (65 used, 16B-aligned pairs)

MAGIC_RECIP = 0x7EF311C3
MAGIC_RSQRT = 0x5F3759DF


def build_program():
    nc = bacc.Bacc("TRN2", target_bir_lowering=False, debug=False)

    q_d = nc.dram_tensor("q", [SQ, DIN], FP32, kind="ExternalInput")
    k_d = nc.dram_tensor("k", [S, DIN], FP32, kind="ExternalInput")
    v_d = nc.dram_tensor("v", [S, DIN], FP32, kind="ExternalInput")
    wq_d = nc.dram_tensor("wq", [DIN, DM], FP32, kind="ExternalInput")
    wk_d = nc.dram_tensor("wk", [DIN, DM], FP32, kind="ExternalInput")
    wv_d = nc.dram_tensor("wv", [DIN, DM], FP32, kind="ExternalInput")
    wo_d = nc.dram_tensor("wo", [DM, DIN], FP32, kind="ExternalInput")
    out_d = nc.dram_tensor("out", [SQ, DIN], FP32, kind="ExternalOutput")

    with tile.TileContext(nc) as tc, ExitStack() as ctx:
        const = ctx.enter_context(tc.tile_pool(name="const", bufs=1))
        wpool = ctx.enter_context(tc.tile_pool(name="wpool", bufs=1))
        resid = ctx.enter_context(tc.tile_pool(name="resid", bufs=1))
        xTp = ctx.enter_context(tc.tile_pool(name="xTp", bufs=1))
        projp = ctx.enter_context(tc.tile_pool(name="projp", bufs=1))
        attnp = ctx.enter_context(tc.tile_pool(name="attnp", bufs=1))
        epool = ctx.enter_context(tc.tile_pool(name="epool", bufs=4))
        outp = ctx.enter_context(tc.tile_pool(name="outp", bufs=3))

        # --- constants ---
        ident_bf = const.tile([128, 128], BF16, tag="identbf")
        make_identity(nc, ident_bf[:])
        cexp_t = const.tile([128, 1], FP32, tag="cexp")
        nc.gpsimd.memset(cexp_t[:], -C_EXP)
        ones64 = const.tile([64, 64], BF16, tag="ones64")
        nc.gpsimd.memset(ones64[:], 1.0)

        # --- phase 0: load, cast bf16, PE-transpose, store fp8 ---
        # transposed inputs, fp8, [i-part(128), ic, tokens]
        qT8 = xTp.tile([128, NIC, SQ], FP8, tag="qT8")
        kT8 = xTp.tile([128, NIC, S], FP8, tag="kT8")
        vT8 = xTp.tile([128, NIC, S], FP8, tag="vT8")
        q_all = resid.tile([128, NT_Q, DIN], FP32, tag="qresid")

        w8 = {}
        ps2 = ctx.enter_context(tc.tile_pool(name="ps2", bufs=2, space="PSUM"))
        phase2 = ExitStack()
        pso = phase2.enter_context(
            tc.tile_pool(name="pso", bufs=1, space="PSUM"))
        psr_box = []
        phase1 = ExitStack()
        stage = phase1.enter_context(tc.tile_pool(name="stage", bufs=1))
        phase1a = ExitStack()
        stage1 = phase1a.enter_context(tc.tile_pool(name="stage1", bufs=1))
        pt_ps = phase1a.enter_context(
            tc.tile_pool(name="ptps", bufs=2, space="PSUM"))
        psj_box = []

        def load_w8(name, wd):
            wst = stage.tile([128, NIC, DM], FP32, tag="wst", bufs=1,
                             name=f"{name}st")
            nc.sync.dma_start(
                wst[:], wd[:, :].rearrange("(ic p) d -> p ic d", p=128))
            wb = wpool.tile([128, NIC, DM], FP8, tag=f"{name}8",
                            name=f"{name}8")
            nc.vector.tensor_copy(wb[:], wst[:])
            w8[name] = wb

        def trans_tiles(src_bf, dst8, tt0, ntt, src0=0):
            # src_bf [128, ntt, 512] bf16 token tiles -> dst8 [128, NIC, S]
            for t in range(ntt):
                pt = pt_ps.tile([128, NIC, 128], BF16, tag="pt", name="pt")
                for ic in range(NIC):
                    nc.tensor.transpose(
                        pt[:, ic, :],
                        src_bf[:, src0 + t, ic * 128:(ic + 1) * 128],
                        ident_bf[:])
                nc.vector.tensor_copy(
                    dst8[:, :, (tt0 + t) * 128:(tt0 + t + 1) * 128], pt[:])

        # dk-split DoubleRow layouts: [32, head, dk-half, tokens]
        QT8i = projp.tile([32, H, 2, SQ], FP8, tag="QT8i")
        KT8i = projp.tile([32, H, 2, S], FP8, tag="KT8i")
        Vt8 = projp.tile([128, NT_K, H, VW], FP8, tag="Vt8")
        nc.gpsimd.memset(Vt8[:, :, :, DK:DK + 1], 1.0)

        def proj_half(w8t, rhs8, dc, tok0):
            # one [128, 512] psum half of a DoubleRow projection
            pj = proj_psum()
            for j in range(2):
                nc.tensor.matmul(
                    pj[:], w8t[:, 2 * j:2 * j + 2, dc * 128:(dc + 1) * 128],
                    rhs8[:, 2 * j:2 * j + 2, tok0:tok0 + 512],
                    start=(j == 0), stop=(j == 1), perf_mode=DR)
            return pj

        def proj_psum():
            if psj_box:
                return psj_box[0].tile([128, 512], FP32, tag="psj",
                                       bufs=2, name="pj")
            return ps2.tile([128, 1024], FP32, tag="ps2", name="pj")[:, 0:512]

        def k_proj_kb(dc, kb, kstg):
            for n in range(2):
                pj = proj_half(w8["wk"], kT8, dc, kb * 1024 + n * 512)
                nc.vector.tensor_copy(
                    kstg[:, kb * 1024 + n * 512:kb * 1024 + (n + 1) * 512],
                    pj[:])

        def k_proj_fin(dc, kstg):
            for hh in range(2):
                for r in range(2):
                    nc.sync.dma_start(
                        KT8i[:, 2 * dc + hh, r, :],
                        kstg[hh * 64 + r * 32:hh * 64 + (r + 1) * 32, :])

        def k_proj(dc):
            kstg = stage.tile([128, S], FP8, tag="kstg", bufs=2, name="kstg")
            for kb in range(2):
                k_proj_kb(dc, kb, kstg)
            k_proj_fin(dc, kstg)

        def q_proj(dc):
            qstg = stage.tile([128, SQ], FP8, tag="qstg", bufs=2, name="qstg")
            for n in range(2):
                pj = proj_half(w8["wq"], qT8, dc, n * 512)
                nc.vector.tensor_copy(
                    qstg[:, n * 512:(n + 1) * 512], pj[:])
            for hh in range(2):
                for r in range(2):
                    nc.sync.dma_start(
                        QT8i[:, 2 * dc + hh, r, :],
                        qstg[hh * 64 + r * 32:hh * 64 + (r + 1) * 32, :])

        def kq_proj(dc):
            k_proj(dc)
            q_proj(dc)

        def v_proj(tt):
            # V natural [tokens, dm] into the per-head 72-padded layout
            pv = proj_psum()
            for j in range(2):
                nc.tensor.matmul(
                    pv[:], vT8[:, 2 * j:2 * j + 2, tt * 128:(tt + 1) * 128],
                    w8["wv"][:, 2 * j:2 * j + 2, :],
                    start=(j == 0), stop=(j == 1), perf_mode=DR)
            nc.vector.tensor_copy(
                Vt8[:, tt, :, 0:DK],
                pv.rearrange("p (h d) -> p h d", d=DK))

        # k chain first: K-projection heads the attention pipeline
        load_w8("wk", wk_d)
        kbf_chunks = []
        for c in range(4):
            rows = slice(c * 4 * 128, (c + 1) * 4 * 128)
            ldc = stage1.tile([128, 4, DIN], FP32, tag="ldc", bufs=3,
                             name=f"kld{c}")
            nc.sync.dma_start(
                ldc[:], k_d[rows, :].rearrange("(tt p) i -> p tt i", p=128))
            cbf = stage1.tile([128, 4, DIN], BF16, tag="cbf", bufs=3,
                             name=f"kbf{c}")
            nc.vector.tensor_copy(cbf[:], ldc[:])
            trans_tiles(cbf, kT8, c * 4, 4)
        # q chain: q DMA queued right after k so the first scores fire early
        load_w8("wq", wq_d)
        for qh in range(2):
            rows = slice(qh * 4 * 128, (qh + 1) * 4 * 128)
            nc.sync.dma_start(
                q_all[:, qh * 4:(qh + 1) * 4, :],
                q_d[rows, :].rearrange("(tt p) i -> p tt i", p=128))
        # v DMAs after q in the queues
        load_w8("wv", wv_d)
        vlds = []
        for c in range(4):
            rows = slice(c * 4 * 128, (c + 1) * 4 * 128)
            vld = stage1.tile([128, 4, DIN], FP32, tag="ldc", bufs=3,
                              name=f"vld{c}")
            nc.sync.dma_start(
                vld[:], v_d[rows, :].rearrange("(tt p) i -> p tt i", p=128))
            vlds.append(vld)
        k_proj(0)
        qbf = stage1.tile([128, NT_Q, DIN], BF16, tag="qbf")
        for qh in range(2):
            nc.vector.tensor_copy(qbf[:, qh * 4:(qh + 1) * 4, :],
                                  q_all[:, qh * 4:(qh + 1) * 4, :])
            trans_tiles(qbf, qT8, qh * 4, 4, src0=qh * 4)
        q_proj(0)

        def v_chain():
            for c in range(4):
                cbf = stage1.tile([128, 4, DIN], BF16, tag="cbf", bufs=3,
                                  name=f"vbf{c}")
                nc.vector.tensor_copy(cbf[:], vlds[c][:])
                trans_tiles(cbf, vT8, c * 4, 4)
            phase1a.close()  # free ldc/cbf/qbf SBUF + transpose PSUM banks
            psj_box.append(phase1.enter_context(
                tc.tile_pool(name="psj", bufs=1, space="PSUM")))

        # deferred phase-0/1 work, drip-fed into the attention loop
        def wo_load():
            wost = stage.tile([128, NDC, DIN], FP32, tag="wst", bufs=1)
            nc.sync.dma_start(
                wost[:], wo_d[:, :].rearrange("(dc p) d -> p dc d", p=128))
            wo8_t = wpool.tile([128, NDC, DIN], FP8, tag="wo8", name="wo8")
            nc.vector.tensor_copy(wo8_t[:], wost[:])
            wo8_box.append(wo8_t)

        wo8_box = []
        pending = []
        for t in range(NT_K):
            pending.append(lambda t=t: v_proj(t))
        for dcx in (1, 2, 3):
            kst_box = {}

            def mk(dcx=dcx, kst_box=kst_box):
                def kb0():
                    kst_box['t'] = stage.tile([128, S], FP8, tag="kstg",
                                              bufs=2, name="kstg")
                    k_proj_kb(dcx, 0, kst_box['t'])

                def kb1():
                    k_proj_kb(dcx, 1, kst_box['t'])
                    k_proj_fin(dcx, kst_box['t'])
                return kb0, kb1
            kb0, kb1 = mk()
            pending.append(kb0)
            pending.append(kb1)
            pending.append(lambda dcx=dcx: q_proj(dcx))
        pending.append(wo_load)

        def close_phase1():
            phase1.close()  # frees stage/psj; then the norm-phase PSUM pool
            psr_box.append(phase2.enter_context(
                tc.tile_pool(name="psr", bufs=1, space="PSUM")))
        pending.append(close_phase1)

        # --- phase 2: attention ---
        # Dsb2: D-row of the pair's first head at partition 0, second at 32
        Dsb2 = attnp.tile([64, SQ], BF16, tag="Dsb2")
        nc.gpsimd.memset(Dsb2[:], 1.0)
        Dt = attnp.tile([128, NT_Q, 2, 32], BF16, tag="Dt")
        Dtf = attnp.tile([128, NT_Q, 2, 1], FP32, tag="Dtf")
        yA = attnp.tile([128, NT_Q, 2, 1], FP32, tag="yA")
        yB = attnp.tile([128, NT_Q, 2, 1], FP32, tag="yB")
        rt_bf = attnp.tile([128, NT_Q, 2, 32], BF16, tag="rtbf")
        rrow2 = attnp.tile([64, NT_Q, 4, 32], BF16, tag="rrow2")
        OT8 = attnp.tile([128, NDC, SQ], FP8, tag="OT8")

        def norm_a(dc):
            # DVE-only: transpose D rows, Newton reciprocal, transpose back
            dsrc = Dsb2.rearrange("p (c im) -> p c im", im=128)
            for X in range(2):
                for i in range(4):
                    nc.vector.transpose(
                        Dt[32 * i:32 * (i + 1), :, X, :],
                        dsrc[32 * X:32 * X + 32, :, 32 * i:32 * (i + 1)])
            sl = (slice(None), slice(None), slice(None), slice(0, 1))
            nc.vector.tensor_copy(Dtf[:], Dt[sl])
            d_i = Dtf.bitcast(I32)
            y_i = yA.bitcast(I32)
            # y0 = bitcast(MAGIC - bits(D)) = bitcast(~bits(D) + MAGIC + 1)
            nc.vector.tensor_scalar(
                out=y_i[:], in0=d_i[:], scalar1=0, op0=OP.bitwise_not,
                scalar2=0, op1=OP.bypass)
            nc.vector.tensor_scalar(
                out=y_i[:], in0=y_i[:], scalar1=MAGIC_RECIP + 1, op0=OP.add,
                scalar2=0, op1=OP.bypass)
            # two Newton steps, tracking m = -y to avoid reverse-subtract
            nc.vector.tensor_tensor(out=yB[:], in0=Dtf[:], in1=yA[:],
                                    op=OP.mult)
            nc.vector.scalar_tensor_tensor(
                out=yB[:], in0=yB[:], scalar=2.0, in1=yA[:],
                op0=OP.subtract, op1=OP.mult)  # m1 = (Dy0-2)y0 = -y1
            nc.vector.tensor_tensor(out=yA[:], in0=Dtf[:], in1=yB[:],
                                    op=OP.mult)  # u2 = -D*y1
            nc.vector.scalar_tensor_tensor(
                out=yB[:], in0=yA[:], scalar=2.0, in1=yB[:],
                op0=OP.add, op1=OP.mult)  # m2 = (2-Dy1)(-y1) = -1/D
            nc.vector.tensor_scalar(
                out=rt_bf[sl], in0=yB[:], scalar1=-1.0, op0=OP.mult,
                scalar2=0.0, op1=OP.bypass)
            for X in range(2):
                for i in range(4):
                    nc.vector.transpose(
                        rrow2[32 * X:32 * X + 32, :, i, :],
                        rt_bf[32 * i:32 * i + 32, :, X, :])

        def norm_b(dc, poSB):
            # PE broadcast of 1/D + the OT multiply (deferred 2 pair-slots)
            rrow_flat = rrow2.rearrange("p c i m -> p (c i m)")
            for hh in range(2):
                oth = attnp.tile([64, SQ], FP8, tag="oth", bufs=2, name="oth")
                for n in range(2):
                    rrep = psr_box[0].tile([64, 512], FP32, tag="psr",
                                           bufs=2, name="rrep")
                    nc.tensor.matmul(
                        rrep[:], ones64[32 * hh:32 * hh + 1, :],
                        rrow_flat[32 * hh:32 * hh + 1,
                                  n * 512:(n + 1) * 512],
                        start=True, stop=True)
                    nc.vector.tensor_tensor(
                        out=oth[:, n * 512:(n + 1) * 512],
                        in0=poSB[hh][0:DK, n * 512:(n + 1) * 512],
                        in1=rrep[:], op=OP.mult)
                nc.sync.dma_start(OT8[hh * 64:(hh + 1) * 64, dc, :], oth[:])

        NPAIR = NT_K // 2  # 8 key-chunk pairs per head
        poSB_pair = [None, None]
        delayed = []   # (due_g, fn)

        def evac_head(ph, ppo, gnow):
            # po PSUM -> SBUF (O rows + D row), D row -> Dsb2 via DMA
            pdc, phh = ph // 2, ph % 2
            poSB = attnp.tile([DK + 1, SQ], BF16, tag="poSB", bufs=2,
                              name="poSB")
            nc.vector.tensor_copy(poSB[:], ppo[:])
            nc.sync.dma_start(Dsb2[32 * phh:32 * phh + 1, :],
                              poSB[DK:DK + 1, :])
            poSB_pair[phh] = poSB
            if phh == 1:
                norm_a(pdc)
                pair = list(poSB_pair)
                delayed.append((gnow + 4, lambda: norm_b(pdc, pair)))

        inflight = []      # [(h, j, e_pair, po)] attnV lags 2 pairs

        def do_av(ph, pj, pe, ppo, gnow):
            for n in range(2):
                nc.tensor.matmul(
                    ppo[:, n * 512:(n + 1) * 512],
                    Vt8[:, 2 * pj:2 * pj + 2, ph, 0:DK + 1],
                    pe[:, :, n * 512:(n + 1) * 512],
                    start=(pj == 0), stop=(pj == NPAIR - 1),
                    perf_mode=DR, skip_group_check=True)
            if pj == NPAIR - 1:
                evac_head(ph, ppo, gnow)

        po = None
        for g in range(H * NPAIR):
            if g == 2:
                v_chain()
            while delayed and delayed[0][0] <= g:
                delayed.pop(0)[1]()
            if g >= 2:
                for _ in range(2):
                    if pending:
                        pending.pop(0)()
            h, j = g // NPAIR, g % NPAIR
            dc, hh = h // 2, h % 2
            if j == 0:
                po = pso.tile([DK + 1, SQ], FP32, tag="pso", name="po")
            e_pair = epool.tile([128, 2, SQ], FP8, tag="e", name="e")
            for sl2 in range(2):
                kc = 2 * j + sl2
                ss = ps2.tile([128, SQ], FP32, tag="ps2", name="ss")
                for n in range(2):
                    nc.tensor.matmul(
                        ss[:, n * 512:(n + 1) * 512],
                        KT8i[:, h, :, kc * 128:(kc + 1) * 128],
                        QT8i[:, h, :, n * 512:(n + 1) * 512],
                        start=True, stop=True, perf_mode=DR)
                nc.scalar.activation(e_pair[:, sl2, :], ss[:], AF.Exp,
                                     bias=cexp_t[:], scale=0.125)
            if len(inflight) >= 2:
                do_av(*inflight.pop(0), g)
            inflight.append((h, j, e_pair, po))
        # drain the remaining pairs
        for item in inflight:
            do_av(*item, 10 ** 9)
        for _, fn in delayed:
            fn()
        delayed = []
        phase2.close()  # free pso/psr PSUM banks before the LN-phase pool

        # --- phase 3: out-projection + residual + LayerNorm ---
        lnp = ctx.enter_context(tc.tile_pool(name="lnp", bufs=1))
        with tc.tile_pool(name="psz", bufs=4, space="PSUM") as psz, \
             tc.tile_pool(name="lns", bufs=2) as lns:
            mvall = lnp.tile([128, NT_Q, 2], FP32, tag="mvall")
            vtmp = lnp.tile([128, NT_Q], FP32, tag="vtmp")
            rsA = lnp.tile([128, NT_Q], FP32, tag="rsA")
            rsB = lnp.tile([128, NT_Q], FP32, tag="rsB")
            x_tiles = []
            for t in range(NT_Q):
                zp = psz.tile([128, DIN], FP32, tag="psz", name="zp")
                for m in range(2):
                    nc.tensor.matmul(
                        zp[:],
                        OT8[:, 2 * m:2 * m + 2, t * 128:(t + 1) * 128],
                        wo8_box[0][:, 2 * m:2 * m + 2, :],
                        start=(m == 0), stop=(m == 1), perf_mode=DR)
                x = lnp.tile([128, DIN], FP32, tag=f"x{t}", name=f"x{t}")
                nc.vector.tensor_tensor(out=x[:], in0=zp[:],
                                        in1=q_all[:, t, :], op=OP.add)
                st = lns.tile([128, 6], FP32, tag="st", name="st")
                nc.vector.bn_stats(st[:], x[:])
                nc.vector.bn_aggr(mvall[:, t, :], st[:])
                x_tiles.append(x)
            # rstd = rsqrt(var + eps) via Quake-Newton on DVE
            nc.vector.tensor_scalar(
                out=vtmp[:], in0=mvall[:, :, 1], scalar1=EPS, op0=OP.add,
                scalar2=0.0, op1=OP.bypass)
            v_i = vtmp.bitcast(I32)
            y_i = rsA.bitcast(I32)
            nc.vector.tensor_scalar(
                out=y_i[:], in0=v_i[:], scalar1=1,
                op0=OP.logical_shift_right, scalar2=0, op1=OP.bitwise_not)
            nc.vector.tensor_scalar(
                out=y_i[:], in0=y_i[:], scalar1=MAGIC_RSQRT + 1, op0=OP.add,
                scalar2=0, op1=OP.bypass)
            for it in range(2):
                nc.vector.tensor_tensor(out=rsB[:], in0=rsA[:], in1=rsA[:],
                                        op=OP.mult)
                nc.vector.tensor_tensor(out=rsB[:], in0=vtmp[:], in1=rsB[:],
                                        op=OP.mult)
                nc.vector.tensor_scalar(
                    out=rsB[:], in0=rsB[:], scalar1=-0.5, op0=OP.mult,
                    scalar2=1.5, op1=OP.add)
                nc.vector.tensor_tensor(out=rsA[:], in0=rsB[:], in1=rsA[:],
                                        op=OP.mult)
            for t in range(NT_Q):
                ot = outp.tile([128, DIN], FP32, tag="oout", name="ot")
                nc.vector.tensor_scalar(
                    out=ot[:], in0=x_tiles[t][:],
                    scalar1=mvall[:, t, 0:1],
                    scalar2=rsA[:, t:t + 1],
                    op0=OP.subtract, op1=OP.mult)
                nc.sync.dma_start(out_d[t * 128:(t + 1) * 128, :], ot[:])

    nc.compile()
    return nc


_PROGRAM = None


def _get_program():
    global _PROGRAM
    if _PROGRAM is None:
        _PROGRAM = build_program()
    return _PROGRAM


def _make_in_maps(q, k, v, Wq, Wk, Wv, Wo):
    in_maps = []
    for c in range(NCORES):
        b, qh = c // 2, c % 2
        in_maps.append({
            "q": np.ascontiguousarray(q[b, qh * SQ:(qh + 1) * SQ, :]),
            "k": np.ascontiguousarray(k[b]),
            "v": np.ascontiguousarray(v[b]),
            "wq": Wq, "wk": Wk, "wv": Wv, "wo": Wo,
        })
    return in_maps


def _assemble(results):
    out = np.empty((B, S, DIN), np.float32)
    for c in range(NCORES):
        b, qh = c // 2, c % 2
        out[b, qh * SQ:(qh + 1) * SQ, :] = results[c]["out"]
    return out


def run(trace=False, **inputs):
    f32 = lambda x: np.asarray(x, dtype=np.float32)
    q, k, v = f32(inputs["q"]), f32(inputs["k"]), f32(inputs["v"])
    Wq, Wk, Wv, Wo = (f32(inputs[n]) for n in ("Wq", "Wk", "Wv", "Wo"))
    nc = _get_program()
    in_maps = _make_in_maps(q, k, v, Wq, Wk, Wv, Wo)
    res = run_bass_kernel_spmd(nc, in_maps, list(range(NCORES)), trace=trace)
    return _assemble(res.results), res.exec_time_ns


def kernel(**inputs):
    out, _ = run(trace=False, **inputs)
    return out
